# revision 2
# baseline (speedup 1.0000x reference)
"""Block-diagonal 2x2 equalizer kernel for Trainium2 (8 NeuronCores), v2.

Per point (b, u, s, f) solves the 2x2 system M x = v by Cramer's rule:
    m_ij = h[b, pi[u], i, 0, 2u+j, s, f]   (only 1/4 of h is needed)
    det  = m00*m11 - m01*m10               (fp32, catastrophic cancellation:
                                            min |det| ~ 1.5e-4 -> h/det chain
                                            MUST stay fp32)
    x0   = (m11*v0 - m01*v1) / det         (numerators tolerate fp16: errors
    x1   = (m00*v1 - m10*v0) / det          are not det-amplified)

Sharding: pure data parallel over batch, 2 batches per core on 8 cores.

v2 design (baseline was 43.2us, all-fp32 DVE-bound at 22.4us):
  - I/O: h planes fp32 (det precision), v planes fp16, x output fp16
    -> 5.5MB/core vs 7.34MB all-fp32.
  - DVE: per chunk 3 fp32 TT ops (p0, p1, det) at 1x rate, then the whole
    numerator chain as fp16 InstTensorScalarPtr ops which support the
    4x_2p DVE perf mode (all operands 2-byte packed SBUF) -> ~9us total.
  - ACT (scalar engine, runs concurrently with DVE): fp32->fp16 converts
    of the m planes, the Reciprocal spline (fp32 in -> fp16 out), and
    issues the output DMAs on its own HWDGE queue.
  - NCH=8 chunks, every SBUF buffer written exactly once (pure dataflow,
    no WAR hazards), all waits standalone wait_ge (walrus single-wait
    rule), cross-engine waits given >=1 chunk of slack to avoid the
    ~1-2us pipe-DRAIN stall on hot semaphores.
  - Block(no_gpsimd_drain=True): skips GpSimd's expensive dge_drain in
    the teardown barrier.

Packing per core ([128, 1792] points, chunked along free dim into
[NCH, 128, FC]): A32 = {m11|m01}, B32 = {m10|m00}, V = {v0|v1} so that
  p0  = A[:, :FC]*B[:, FC:]  (m11*m00)     p1 = A[:, FC:]*B[:, :FC]
  QA  = Af*V = {m11f*v0 | m01f*v1}         r0 = QA[:, :FC]-QA[:, FC:]
  QB  = Bf*V = {m10f*v0 | m00f*v1}         r1 = QB[:, FC:]-QB[:, :FC]
  x0  = r0*rdet,  x1 = r1*rdet             X  = {x0|x1}
"""

from contextlib import ExitStack

import numpy as np

import concourse.bass as bass
import concourse.mybir as mybir
from concourse.bass_utils import run_bass_kernel_spmd

# Problem shapes (hardcoded per contract)
B, U, A, NTX, T, S, F = 16, 4, 2, 1, 8, 14, 2048
SF = S * F               # 28672
NCORES = 8
BPC = B // NCORES        # 2 batches per core
PTS = BPC * U * SF       # 229376 points per core
COLS = PTS // 128        # 1792 free elems per partition
NCH = 8                  # pipeline chunks
FC = COLS // NCH         # 224 free elems per chunk per component
WC = 2 * FC              # 448 (two components side by side)

# Set by test harness to capture an NTFF profile on the run.
TRACE = False
LAST_RESULTS = None

f32 = mybir.dt.float32
f16 = mybir.dt.float16
MUL = mybir.AluOpType.mult
SUB = mybir.AluOpType.subtract


def _build_nc():
    nc = bass.Bass("TRN2")
    hA = nc.dram_tensor("hA", [NCH, 128, WC], f32, kind="ExternalInput")
    hB = nc.dram_tensor("hB", [NCH, 128, WC], f32, kind="ExternalInput")
    yV = nc.dram_tensor("yV", [NCH, 128, WC], f16, kind="ExternalInput")
    xO = nc.dram_tensor("xO", [NCH, 128, WC], f16, kind="ExternalOutput")

    with ExitStack() as ctx:
        sb = lambda name, w, dt: ctx.enter_context(nc.sbuf_tensor(name, [128, w], dt))
        tA = [sb(f"tA{k}", WC, f32) for k in range(NCH)]
        tB = [sb(f"tB{k}", WC, f32) for k in range(NCH)]
        tV = [sb(f"tV{k}", WC, f16) for k in range(NCH)]
        tAf = [sb(f"tAf{k}", WC, f16) for k in range(NCH)]
        tBf = [sb(f"tBf{k}", WC, f16) for k in range(NCH)]
        tP0 = [sb(f"tP0{k}", FC, f32) for k in range(NCH)]
        tP1 = [sb(f"tP1{k}", FC, f32) for k in range(NCH)]
        tDet = [sb(f"tDet{k}", FC, f32) for k in range(NCH)]
        tRd = [sb(f"tRd{k}", FC, f16) for k in range(NCH)]
        tQA = [sb(f"tQA{k}", WC, f16) for k in range(NCH)]
        tQB = [sb(f"tQB{k}", WC, f16) for k in range(NCH)]
        tR0 = [sb(f"tR0{k}", FC, f16) for k in range(NCH)]
        tR1 = [sb(f"tR1{k}", FC, f16) for k in range(NCH)]
        tX = [sb(f"tX{k}", WC, f16) for k in range(NCH)]

        inS = [ctx.enter_context(nc.semaphore(f"inS{k}")) for k in range(NCH)]
        dveS = ctx.enter_context(nc.semaphore("dveS"))
        actS = ctx.enter_context(nc.semaphore("actS"))
        outS = ctx.enter_context(nc.semaphore("outS"))

        # semaphore count bookkeeping (indices are 1-based thresholds)
        det_idx = [0] * NCH    # dveS value after det(k)
        x_idx = [0] * NCH      # dveS value after x1(k)
        recip_idx = [0] * NCH  # actS value after recip(k)

        # precompute schedule indices
        dc = 0
        ac = 0
        for t in range(NCH + 2):
            if t < NCH:
                dc += 3
                det_idx[t] = dc
            if t >= 2:
                dc += 6
                x_idx[t - 2] = dc
        for t in range(NCH + 3):
            if t >= 1 and t - 1 < NCH:
                ac += 1
                recip_idx[t - 1] = ac
            if t < NCH:
                ac += 2

        with nc.Block(no_gpsimd_drain=True) as block:

            @block.sync
            def _(sync):
                for k in range(NCH):
                    sync.dma_start(out=tA[k][:], in_=hA[k]).then_inc(inS[k], 16)
                    sync.dma_start(out=tB[k][:], in_=hB[k]).then_inc(inS[k], 16)
                    sync.dma_start(out=tV[k][:], in_=yV[k]).then_inc(inS[k], 16)
                sync.wait_ge(outS, NCH * 16)

            @block.vector
            def _(vector):
                for t in range(NCH + 2):
                    if t < NCH:
                        a, b = tA[t], tB[t]
                        vector.wait_ge(inS[t], 48)
                        vector.tensor_mul(tP0[t][:], a[:, :FC], b[:, FC:]).then_inc(dveS, 1)
                        vector.tensor_mul(tP1[t][:], a[:, FC:], b[:, :FC]).then_inc(dveS, 1)
                        vector.tensor_sub(tDet[t][:], tP0[t][:], tP1[t][:]).then_inc(dveS, 1)
                    if t >= 2:
                        k = t - 2
                        vector.wait_ge(actS, recip_idx[k])
                        vector.scalar_tensor_tensor(
                            tQA[k][:], tAf[k][:], 1.0, tV[k][:], MUL, MUL
                        ).then_inc(dveS, 1)
                        vector.scalar_tensor_tensor(
                            tQB[k][:], tBf[k][:], 1.0, tV[k][:], MUL, MUL
                        ).then_inc(dveS, 1)
                        vector.scalar_tensor_tensor(
                            tR0[k][:], tQA[k][:, :FC], 1.0, tQA[k][:, FC:], MUL, SUB
                        ).then_inc(dveS, 1)
                        vector.scalar_tensor_tensor(
                            tR1[k][:], tQB[k][:, FC:], 1.0, tQB[k][:, :FC], MUL, SUB
                        ).then_inc(dveS, 1)
                        vector.scalar_tensor_tensor(
                            tX[k][:, :FC], tR0[k][:], 1.0, tRd[k][:], MUL, MUL
                        ).then_inc(dveS, 1)
                        vector.scalar_tensor_tensor(
                            tX[k][:, FC:], tR1[k][:], 1.0, tRd[k][:], MUL, MUL
                        ).then_inc(dveS, 1)

            @block.scalar
            def _(scalar):
                for t in range(NCH + 3):
                    if t >= 3 and t - 3 < NCH:
                        k = t - 3
                        scalar.wait_ge(dveS, x_idx[k])
                        scalar.dma_start(out=xO[k], in_=tX[k][:]).then_inc(outS, 16)
                    if t >= 1 and t - 1 < NCH:
                        k = t - 1
                        scalar.wait_ge(dveS, det_idx[k])
                        scalar.add_instruction(
                            mybir.InstActivation(
                                name=nc.get_next_instruction_name(),
                                func=mybir.ActivationFunctionType.Reciprocal,
                                ins=[
                                    scalar.lower_ap(tDet[k][:]),
                                    mybir.ImmediateValue(dtype=f32, value=0.0),
                                    mybir.ImmediateValue(dtype=f32, value=1.0),
                                    mybir.ImmediateValue(dtype=f32, value=0.0),
                                ],
                                outs=[scalar.lower_ap(tRd[k][:])],
                            )
                        ).then_inc(actS, 1)
                    if t < NCH:
                        scalar.wait_ge(inS[t], 48)
                        scalar.copy(tAf[t][:], tA[t][:]).then_inc(actS, 1)
                        scalar.copy(tBf[t][:], tB[t][:]).then_inc(actS, 1)

    return nc


def _chunk(plane):
    """[128*COLS] flat (C-order over [BPC,U,S,F]) -> [NCH, 128, FC]."""
    return plane.reshape(128, NCH, FC).transpose(1, 0, 2)


def make_in_maps(y, h, precoding_ind):
    """Host-side gather + pack. Returns per-core input maps."""
    y = np.asarray(y)
    h = np.asarray(h)
    pi = np.asarray(precoding_ind).astype(np.int64)

    hg = h[:, pi[0]]                                     # [B, U, A, NTX, T, S, F]
    # hsel[b, u, i, j] = hg[b, u, i, 0, 2u+j]
    hsel = np.stack(
        [hg[:, u, :, 0, 2 * u:2 * u + 2] for u in range(U)], axis=1
    )                                                    # [B, U, A(i), 2(j), S, F]
    hsel = np.ascontiguousarray(hsel).astype(np.float32)
    yr = np.ascontiguousarray(y).astype(np.float32)      # [B, U, A, S, F]

    in_maps = []
    for c in range(NCORES):
        b0 = c * BPC
        hs = hsel[b0:b0 + BPC]                           # [BPC, U, 2, 2, S, F]
        ys = yr[b0:b0 + BPC]                             # [BPC, U, A, S, F]
        m00 = np.ascontiguousarray(hs[:, :, 0, 0]).reshape(-1)
        m01 = np.ascontiguousarray(hs[:, :, 0, 1]).reshape(-1)
        m10 = np.ascontiguousarray(hs[:, :, 1, 0]).reshape(-1)
        m11 = np.ascontiguousarray(hs[:, :, 1, 1]).reshape(-1)
        v0 = np.ascontiguousarray(ys[:, :, 0]).reshape(-1)
        v1 = np.ascontiguousarray(ys[:, :, 1]).reshape(-1)
        hA = np.concatenate([_chunk(m11), _chunk(m01)], axis=2)
        hB = np.concatenate([_chunk(m10), _chunk(m00)], axis=2)
        yV = np.concatenate([_chunk(v0), _chunk(v1)], axis=2).astype(np.float16)
        in_maps.append({
            "hA": np.ascontiguousarray(hA),
            "hB": np.ascontiguousarray(hB),
            "yV": np.ascontiguousarray(yV),
        })
    return in_maps


def _unchunk(t):
    """[NCH, 128, FC] -> [128*COLS] flat."""
    return t.transpose(1, 0, 2).reshape(-1)


def assemble_output(results):
    """Per-core xO [NCH, 128, WC] f16 -> full [B, U, A, S, F] f32."""
    out = np.empty((B, U, A, S, F), np.float32)
    for c in range(NCORES):
        xo = np.asarray(results[c]["xO"]).astype(np.float32)
        x0 = _unchunk(xo[:, :, :FC]).reshape(BPC, U, S, F)
        x1 = _unchunk(xo[:, :, FC:]).reshape(BPC, U, S, F)
        out[c * BPC:(c + 1) * BPC, :, 0] = x0
        out[c * BPC:(c + 1) * BPC, :, 1] = x1
    return out


def kernel(y, h, precoding_ind):
    global LAST_RESULTS
    in_maps = make_in_maps(y, h, precoding_ind)
    nc = _build_nc()
    res = run_bass_kernel_spmd(nc, in_maps, list(range(NCORES)), trace=TRACE)
    LAST_RESULTS = res
    return assemble_output(res.results)


# revision 3
# speedup vs baseline: 1.2414x; 1.2414x over previous
"""Block-diagonal 2x2 equalizer kernel for Trainium2 (8 NeuronCores), v2.1.

Per point (b, u, s, f) solves the 2x2 system M x = v by Cramer's rule:
    m_ij = h[b, pi[u], i, 0, 2u+j, s, f]   (only 1/4 of h is needed)
    det  = m00*m11 - m01*m10               (fp32: min |det| ~ 1.5e-4, the
                                            det chain MUST stay fp32)
    x0   = (m11*v0 - m01*v1) / det         (numerators tolerate fp16)
    x1   = (m00*v1 - m10*v0) / det

Sharding: pure data parallel over batch, 2 batches per core on 8 cores.

Design (baseline all-fp32 was DVE-bound at 22.4us busy, 43.2us total):
  - I/O: h planes fp32, v planes fp16, x output fp16 -> 5.5MB/core.
  - One byte-packed DMA per chunk ({A fp32|B fp32|V fp16} as uint8, viewed
    via bitcast slices) - a dma_start costs ~600ns of issuing-sequencer
    time, so fewer+bigger DMAs matter; 4480B descriptor rows.
  - DVE: 3 narrow fp32 TT ops (p0, p1, det) at 1x + 5 fp16 TT ops that
    qualify for the HW-auto-detected 2x_1p mode (16-bit dtype, step 1,
    4B-aligned; plain TensorTensor has a 2x_1p uop program, the fused
    TensorScalarPtr does NOT - measured 1x).
  - ACT (parallel scalar engine): fp32->fp16 converts of the m planes,
    Reciprocal spline (fp32 in -> fp16 out), output DMA issue.
  - Pure dataflow: every SBUF region written exactly once; standalone
    wait_ge only; cross-engine waits get >=2 chunks of slack.
  - Block(no_gpsimd_drain=True) to skip the slow gpsimd dge_drain.

Packing per core ([128, 1792] points, chunked into [NCH, 128, FC]):
  A = {m11|m01}, B = {m10|m00}, V = {v0|v1}
    p0 = A[:, :FC]*B[:, FC:]      p1 = A[:, FC:]*B[:, :FC]   (fp32)
    QA = Af*V = {q0|q1} = {m11f*v0 | m01f*v1}
    QB = Bf*V = {q3|q2} = {m10f*v0 | m00f*v1}   -> tQ = {q0|q1|q3|q2}
    R  = {q0|q2} - {q1|q3} = {r0|r1}   (strided wide sub)
    x0 = r0*rdet, x1 = r1*rdet
"""

from contextlib import ExitStack

import numpy as np

import concourse.bass as bass
import concourse.mybir as mybir
from concourse.bass_utils import run_bass_kernel_spmd

# Problem shapes (hardcoded per contract)
B, U, A, NTX, T, S, F = 16, 4, 2, 1, 8, 14, 2048
SF = S * F               # 28672
NCORES = 8
BPC = B // NCORES        # 2 batches per core
PTS = BPC * U * SF       # 229376 points per core
COLS = PTS // 128        # 1792 free elems per partition
NCH = 8                  # pipeline chunks
FC = COLS // NCH         # 224
WC = 2 * FC              # 448
ROW = 2 * WC * 4 + WC * 2  # input bytes per partition per chunk: A,B fp32 + V fp16
AOFF, BOFF, VOFF = 0, WC * 4, 2 * WC * 4
NST = 4                  # output stores (chunk pairs)
D = 3                    # DVE fp16 lag (chunks)

TRACE = False
LAST_RESULTS = None

f32 = mybir.dt.float32
f16 = mybir.dt.float16
u8 = mybir.dt.uint8


def _build_nc():
    nc = bass.Bass("TRN2")
    dIn = nc.dram_tensor("dIn", [NCH, 128, ROW], u8, kind="ExternalInput")
    xO = nc.dram_tensor("xO", [NST, 128, 2 * WC], f16, kind="ExternalOutput")

    with ExitStack() as ctx:
        sb = lambda name, w, dt: ctx.enter_context(nc.sbuf_tensor(name, [128, w], dt))
        tIn = [sb(f"tIn{k}", ROW, u8) for k in range(NCH)]
        tAf = [sb(f"tAf{k}", WC, f16) for k in range(NCH)]
        tBf = [sb(f"tBf{k}", WC, f16) for k in range(NCH)]
        tP0 = [sb(f"tP0{k}", FC, f32) for k in range(NCH)]
        tP1 = [sb(f"tP1{k}", FC, f32) for k in range(NCH)]
        tDet = [sb(f"tDet{k}", FC, f32) for k in range(NCH)]
        tRd = [sb(f"tRd{k}", FC, f16) for k in range(NCH)]
        tQ = [sb(f"tQ{k}", 2 * WC, f16) for k in range(NCH)]
        tR = [sb(f"tR{k}", WC, f16) for k in range(NCH)]
        tX = sb("tX", NCH * WC, f16)

        # views into the byte-packed input tile
        vA = [tIn[k][:, AOFF:BOFF].bitcast(f32) for k in range(NCH)]
        vB = [tIn[k][:, BOFF:VOFF].bitcast(f32) for k in range(NCH)]
        vV = [tIn[k][:, VOFF:ROW].bitcast(f16) for k in range(NCH)]

        inS = [ctx.enter_context(nc.semaphore(f"inS{k}")) for k in range(NCH)]
        dveS = ctx.enter_context(nc.semaphore("dveS"))
        actS = ctx.enter_context(nc.semaphore("actS"))
        outS = ctx.enter_context(nc.semaphore("outS"))

        # schedule indices (1-based semaphore thresholds)
        det_idx = [0] * NCH
        x_idx = [0] * NCH
        recip_idx = [0] * NCH
        dc = 0
        for t in range(NCH + D):
            if t < NCH:
                dc += 3
                det_idx[t] = dc
            if t >= D:
                dc += 5
                x_idx[t - D] = dc
        ac = 0
        for t in range(NCH + 1):
            if 1 <= t <= NCH:
                ac += 1
                recip_idx[t - 1] = ac
            if t < NCH:
                ac += 2

        with nc.Block(no_gpsimd_drain=True) as block:

            @block.sync
            def _(sync):
                for k in range(NCH):
                    sync.dma_start(out=tIn[k][:], in_=dIn[k]).then_inc(inS[k], 16)
                sync.wait_ge(outS, NST * 16)

            @block.vector
            def _(vector):
                for t in range(NCH + D):
                    if t < NCH:
                        vector.wait_ge(inS[t], 16)
                        vector.tensor_mul(
                            tP0[t][:], vA[t][:, :FC], vB[t][:, FC:]
                        ).then_inc(dveS, 1)
                        vector.tensor_mul(
                            tP1[t][:], vA[t][:, FC:], vB[t][:, :FC]
                        ).then_inc(dveS, 1)
                        vector.tensor_sub(tDet[t][:], tP0[t][:], tP1[t][:]).then_inc(
                            dveS, 1
                        )
                    if t >= D:
                        k = t - D
                        q = tQ[k][:]
                        vector.wait_ge(actS, recip_idx[k])
                        vector.tensor_mul(q[:, :WC], tAf[k][:], vV[k]).then_inc(dveS, 1)
                        vector.tensor_mul(q[:, WC:], tBf[k][:], vV[k]).then_inc(dveS, 1)
                        q4 = q.rearrange("p (a c) -> p a c", a=4, c=FC)
                        rr = tR[k][:].rearrange("p (a c) -> p a c", a=2, c=FC)
                        vector.tensor_sub(rr, q4[:, 0::3], q4[:, 1:3]).then_inc(dveS, 1)
                        vector.tensor_mul(
                            tX[:, k * WC:k * WC + FC], tR[k][:, :FC], tRd[k][:]
                        ).then_inc(dveS, 1)
                        vector.tensor_mul(
                            tX[:, k * WC + FC:(k + 1) * WC], tR[k][:, FC:], tRd[k][:]
                        ).then_inc(dveS, 1)

            @block.scalar
            def _(scalar):
                for t in range(NCH + 5):
                    if t >= 6 and (t - 6) % 2 == 0 and (t - 6) // 2 < NST:
                        p = (t - 6) // 2
                        scalar.wait_ge(dveS, x_idx[2 * p + 1])
                        scalar.dma_start(
                            out=xO[p], in_=tX[:, p * 2 * WC:(p + 1) * 2 * WC]
                        ).then_inc(outS, 16)
                    if 1 <= t <= NCH:
                        k = t - 1
                        scalar.wait_ge(dveS, det_idx[k])
                        scalar.add_instruction(
                            mybir.InstActivation(
                                name=nc.get_next_instruction_name(),
                                func=mybir.ActivationFunctionType.Reciprocal,
                                ins=[
                                    scalar.lower_ap(tDet[k][:]),
                                    mybir.ImmediateValue(dtype=f32, value=0.0),
                                    mybir.ImmediateValue(dtype=f32, value=1.0),
                                    mybir.ImmediateValue(dtype=f32, value=0.0),
                                ],
                                outs=[scalar.lower_ap(tRd[k][:])],
                            )
                        ).then_inc(actS, 1)
                    if t < NCH:
                        scalar.wait_ge(inS[t], 16)
                        scalar.copy(tAf[t][:], vA[t]).then_inc(actS, 1)
                        scalar.copy(tBf[t][:], vB[t]).then_inc(actS, 1)

    return nc


def _chunk(plane):
    """[128*COLS] flat (C-order over [BPC,U,S,F]) -> [NCH, 128, FC]."""
    return plane.reshape(128, NCH, FC).transpose(1, 0, 2)


def make_in_maps(y, h, precoding_ind):
    """Host-side gather + byte-pack. Returns per-core input maps."""
    y = np.asarray(y)
    h = np.asarray(h)
    pi = np.asarray(precoding_ind).astype(np.int64)

    hg = h[:, pi[0]]                                     # [B, U, A, NTX, T, S, F]
    hsel = np.stack(
        [hg[:, u, :, 0, 2 * u:2 * u + 2] for u in range(U)], axis=1
    )                                                    # [B, U, A(i), 2(j), S, F]
    hsel = np.ascontiguousarray(hsel).astype(np.float32)
    yr = np.ascontiguousarray(y).astype(np.float32)      # [B, U, A, S, F]

    in_maps = []
    for c in range(NCORES):
        b0 = c * BPC
        hs = hsel[b0:b0 + BPC]
        ys = yr[b0:b0 + BPC]
        m00 = np.ascontiguousarray(hs[:, :, 0, 0]).reshape(-1)
        m01 = np.ascontiguousarray(hs[:, :, 0, 1]).reshape(-1)
        m10 = np.ascontiguousarray(hs[:, :, 1, 0]).reshape(-1)
        m11 = np.ascontiguousarray(hs[:, :, 1, 1]).reshape(-1)
        v0 = np.ascontiguousarray(ys[:, :, 0]).reshape(-1)
        v1 = np.ascontiguousarray(ys[:, :, 1]).reshape(-1)
        hA = np.concatenate([_chunk(m11), _chunk(m01)], axis=2)  # [NCH,128,WC] f32
        hB = np.concatenate([_chunk(m10), _chunk(m00)], axis=2)
        yV = np.concatenate([_chunk(v0), _chunk(v1)], axis=2).astype(np.float16)
        dIn = np.concatenate(
            [
                hA.view(np.uint8).reshape(NCH, 128, WC * 4),
                hB.view(np.uint8).reshape(NCH, 128, WC * 4),
                yV.view(np.uint8).reshape(NCH, 128, WC * 2),
            ],
            axis=2,
        )
        in_maps.append({"dIn": np.ascontiguousarray(dIn)})
    return in_maps


def _unchunk(t):
    """[NCH, 128, FC] -> [128*COLS] flat."""
    return t.transpose(1, 0, 2).reshape(-1)


def assemble_output(results):
    """Per-core xO [NST, 128, 2*WC] f16 -> full [B, U, A, S, F] f32."""
    out = np.empty((B, U, A, S, F), np.float32)
    for c in range(NCORES):
        xo = np.asarray(results[c]["xO"]).astype(np.float32)
        xo = xo.reshape(NST, 128, 2, WC).transpose(0, 2, 1, 3).reshape(NCH, 128, WC)
        x0 = _unchunk(xo[:, :, :FC]).reshape(BPC, U, S, F)
        x1 = _unchunk(xo[:, :, FC:]).reshape(BPC, U, S, F)
        out[c * BPC:(c + 1) * BPC, :, 0] = x0
        out[c * BPC:(c + 1) * BPC, :, 1] = x1
    return out


def kernel(y, h, precoding_ind):
    global LAST_RESULTS
    in_maps = make_in_maps(y, h, precoding_ind)
    nc = _build_nc()
    res = run_bass_kernel_spmd(nc, in_maps, list(range(NCORES)), trace=TRACE)
    LAST_RESULTS = res
    return assemble_output(res.results)
